# revision 12
# baseline (speedup 1.0000x reference)
"""Trainium2 Bass kernel for nn_Attention_p_2757369004155.

Reference math (per p in 0..4):
  x [256, 1728] -> qkv = W_qkv @ x -> 8 heads of dim 32, N=1728
  attn = softmax((q*scale)^T k), out = v @ attn^T, y = W_p @ out + b

Sharding: 8 cores = 4 p-branches x 2 query-halves. Each core is fully
self-contained (K/V computed for all n, Q for its half). The host permutes
each core's n axis so its query half is always columns [0, 864).

v2 design (cost-model-driven):
  - All matmul MOVING operands are bf16 (1 cyc/row vs fp32's 4). Stationary
    loads are free in the cost model, so A@V uses exp(S^T) tiles as the
    stationary operand and streams only 33 columns (32 v-dims + ones column
    for the softmax denominator) per (head, m-tile).
  - S^T[n, m] = K^T Q per head as single 32-deep bf16 matmuls, sliced
    directly out of the K/Q SBUF tiles at partition offset 32h (no
    stacking DMAs, no f32r split).
  - ACT (scalar engine) does ONLY exp: one 864-col call per (head, nt)
    psum tile. This is the roofline engine (~12M exps/core).
  - A tunable share of exp tiles is offloaded to DVE/Pool via a one-op
    Schraudolph fast-exp: int16(A*x + B) bit-cast to bf16. |rel err| ~3%
    sawtooth on those tiles only; softmax-consistent (denominator uses the
    same approximated weights), so end-to-end error stays ~1e-3..1e-2.
  - PSUM: 3 rotating [128, 2, 512] S^T tiles (6 banks) + 2 single-bank
    A@V accumulators [128, 2, 7, 33]. QKV/proj psum reuses the S^T tag.
  - O^T -> O via DMA transpose (SP queue, async) instead of PE transposes.
  - Output projection + bias on ACT at the tail when it is idle.
"""

import numpy as np

import concourse.bass as bass
import concourse.tile as tile
from concourse import bacc, mybir
from concourse.bass import ds
from concourse.bass_utils import run_bass_kernel_spmd
from concourse.masks import make_identity

F32 = mybir.dt.float32
BF16 = mybir.dt.bfloat16
I16 = mybir.dt.int16
AF = mybir.ActivationFunctionType
ALU = mybir.AluOpType

N_CORES = 8
C = 256            # channels
NH = 8             # heads
HD = 32            # head dim
N = 1728           # sequence (12*12*12)
M = N // 2         # per-core query positions
MC = 432           # m chunk (psum bank = 512 fp32)
NT_SIZES = [128] * 13 + [64]          # n contraction tiles
MT_SIZES = [128] * 6 + [96]           # m tiles for the A@V stationary
SCALE = HD ** -0.5

# Schraudolph one-op fast-exp constants for bf16 bit-trick:
#   bf16_bits(exp(x)) ~= int16(A16 * x + B16)
# A16 = 2^7 / ln 2; B16 tuned numerically for min-max relative error on
# x in [-0.8, 0.8] assuming round-to-nearest on the fp32->int16 convert.
EXP_A16 = 184.6650390625
EXP_B16 = 16250.25

# exp work assignment: for each (g, nt) the four heads' tiles go to these
# engines. "a"=ACT exact exp, "d"=DVE fast-exp, "p"=Pool fast-exp.
# Tuned against TimelineSim.
ASSIGN_DEFAULT = "aaaa"


def build_program(assign=None, fastexp=True):
    nc = bacc.Bacc(
        "TRN2",
        target_bir_lowering=False,
        debug=False,
        enable_asserts=False,
        num_devices=N_CORES,
    )

    xf_d = nc.dram_tensor("xf", [C, N], BF16, kind="ExternalInput").ap()
    wq_d = nc.dram_tensor("wqT", [C, C], BF16, kind="ExternalInput").ap()
    wk_d = nc.dram_tensor("wkT", [C, C], BF16, kind="ExternalInput").ap()
    wv_d = nc.dram_tensor("wvT", [C, C], BF16, kind="ExternalInput").ap()
    wp_d = nc.dram_tensor("wpT", [C, C], BF16, kind="ExternalInput").ap()
    b_d = nc.dram_tensor("bias", [C, 1], F32, kind="ExternalInput").ap()
    y_d = nc.dram_tensor("y", [C, M], F32, kind="ExternalOutput").ap()

    xf_r = xf_d.rearrange("(kt p) n -> p kt n", p=128)

    if assign is None:
        assign = {}

    def eng_of(code):
        return {"a": "act", "d": "dve", "p": "pool"}[code]

    with tile.TileContext(nc) as tc:
        with (
            tc.tile_pool(name="persist", bufs=1) as sb,
            tc.tile_pool(name="ps", bufs=1, space="PSUM") as ps,
        ):
            # ---- input DMAs (SP queue; xf chunk 0 + wq first so Q starts early) ----
            wq_sb = sb.tile([128, 2, 2, 128], BF16, tag="wq")
            wk_sb = sb.tile([128, 2, 2, 128], BF16, tag="wk")
            xf_sb = sb.tile([128, 2, N], BF16, tag="xf")
            nc.sync.dma_start(out=wq_sb, in_=wq_d.rearrange("(kt p) (ot o) -> p kt ot o", p=128, o=128))
            nc.sync.dma_start(out=xf_sb[:, :, ds(0, MC)], in_=xf_r[:, :, ds(0, MC)])
            nc.sync.dma_start(out=wk_sb, in_=wk_d.rearrange("(kt p) (ot o) -> p kt ot o", p=128, o=128))
            nc.sync.dma_start(out=xf_sb[:, :, ds(MC, MC)], in_=xf_r[:, :, ds(MC, MC)])
            wv_sb = sb.tile([128, 2, 256], BF16, tag="wv")
            nc.sync.dma_start(out=wv_sb, in_=wv_d.rearrange("(kt p) c -> p kt c", p=128))
            for ch in range(2, 4):
                nc.sync.dma_start(out=xf_sb[:, :, ds(ch * MC, MC)], in_=xf_r[:, :, ds(ch * MC, MC)])
            wp_sb = sb.tile([128, 2, 2, 128], BF16, tag="wp")
            b_sb = sb.tile([128, 2, 1], F32, tag="b")
            nc.gpsimd.dma_start(out=wp_sb, in_=wp_d.rearrange("(kt p) (ot o) -> p kt ot o", p=128, o=128))
            nc.gpsimd.dma_start(out=b_sb, in_=b_d.rearrange("(ot p) one -> p ot one", p=128))

            # trigger the ACT exp table load during the initial DMA window
            warm = sb.tile([128, 1], F32, tag="warm")
            nc.vector.memset(warm, 0.0)
            nc.scalar.activation(warm, warm, AF.Exp)
            ident = sb.tile([128, 128], F32, tag="ident")
            make_identity(nc, ident)

            # ---- persistent SBUF ----
            k_sb = sb.tile([128, 2, N], BF16, tag="k")       # [32h+d, g, n]
            q_sb = sb.tile([128, 2, M], BF16, tag="q")       # [32h+d, g, m]
            # head 3 lives at partition base 96, which matmul APs cannot
            # address (allowed bases: 0/32/64) — DMA-shift it to base 0
            k3_sb = sb.tile([32, 2, N], BF16, tag="k3")
            q3_sb = sb.tile([32, 2, M], BF16, tag="q3")
            vt_sb = sb.tile([128, 14, NH, 33], BF16, tag="vt")  # [n, nt, head, c+1]
            nc.vector.memset(vt_sb[:, :, :, 32:33], 1.0)
            ex_sb = sb.tile([128, 3, 4, M], BF16, tag="ex")  # [n, slot, h-of-g, m]
            on_t = sb.tile([128, 7, C], F32, tag="on_t")     # [m, mt, c]
            on_sb = sb.tile([128, 2, M], BF16, tag="on")     # [c-half, g2, m]
            y_sb = sb.tile([128, 2, M], F32, tag="y")
            wu_in = sb.tile([128, 128], BF16, tag="wu_in")
            nc.vector.memset(wu_in, 0.0)

            def st_tile():
                return ps.tile([128, 2, 512], F32, tag="st", name="st", bufs=3)

            def warm_pe(n_mm, cols=512):
                for _ in range(n_mm):
                    wu = st_tile()
                    nc.tensor.matmul(
                        wu[:, 0, 0:128], lhsT=wu_in, rhs=wu_in,
                        start=True, stop=True,
                    )
                    del wu

            # PE p-state warmup while input DMAs land
            warm_pe(9)

            # ---- Q for both groups (2 chunks each -> one st tile per g) ----
            for g in range(2):
                pq = st_tile()
                for mc in range(2):
                    for kt in range(2):
                        nc.tensor.matmul(
                            pq[:, mc, 0:MC],
                            lhsT=wq_sb[:, kt, g, :],
                            rhs=xf_sb[:, kt, ds(mc * MC, MC)],
                            start=(kt == 0),
                            stop=(kt == 1),
                        )
                nc.vector.tensor_copy(
                    q_sb[:, g, :].rearrange("p (a b) -> p a b", a=2),
                    pq[:, :, 0:MC],
                )
                nc.sync.dma_start(out=q3_sb[:, g, :], in_=q_sb[ds(96, 32), g, :])

            def emit_k_pair(g, half, engine):
                """K for n-range [864*half, 864*half+864), one head group."""
                pk = st_tile()
                for mc in range(2):
                    for kt in range(2):
                        nc.tensor.matmul(
                            pk[:, mc, 0:MC],
                            lhsT=wk_sb[:, kt, g, :],
                            rhs=xf_sb[:, kt, ds(half * M + mc * MC, MC)],
                            start=(kt == 0),
                            stop=(kt == 1),
                        )
                engine.tensor_copy(
                    k_sb[:, g, ds(half * M, M)].rearrange("p (a b) -> p a b", a=2),
                    pk[:, :, 0:MC],
                )
                nc.sync.dma_start(
                    out=k3_sb[:, g, ds(half * M, M)],
                    in_=k_sb[ds(96, 32), g, ds(half * M, M)],
                )

            def emit_vt_pair(nt, engine):
                """V^T tiles for nt and nt+1 (both head groups) in one st tile."""
                pv = st_tile()
                sizes = [NT_SIZES[nt], NT_SIZES[nt + 1] if nt + 1 < 14 else 0]
                for i, w in enumerate(sizes):
                    if w == 0:
                        continue
                    for g in range(2):
                        for kt in range(2):
                            nc.tensor.matmul(
                                pv[:w, i, ds(128 * g, 128)],
                                lhsT=xf_sb[:, kt, ds((nt + i) * 128, w)],
                                rhs=wv_sb[:, kt, ds(128 * g, 128)],
                                start=(kt == 0),
                                stop=(kt == 1),
                            )
                for i, w in enumerate(sizes):
                    if w == 0:
                        continue
                    engine.tensor_copy(
                        vt_sb[:w, nt + i, :, 0:32],
                        pv[:w, i, 0:256].rearrange("p (h c) -> p h c", h=8),
                    )

            # K first half for both groups, then V^T for nt 0,1
            emit_k_pair(0, 0, nc.vector)
            emit_k_pair(1, 0, nc.vector)
            emit_vt_pair(0, nc.gpsimd)

            # ---- attention ----
            for g in range(2):
                ot_ps = [
                    ps.tile([128, 2, 7, 33], F32, tag=f"ot{j}", name=f"ot{j}", bufs=1)
                    for j in range(2)
                ]
                for nt in range(15):
                    # S^T + exp for this nt
                    if nt < 14:
                        w = NT_SIZES[nt]
                        slot = nt % 3
                        codes = assign.get((g, nt), ASSIGN_DEFAULT)
                        for h in range(4):
                            st = st_tile()
                            if h < 3:
                                lk = k_sb[ds(32 * h, 32), g, ds(nt * 128, w)]
                                rq = lambda mc: q_sb[ds(32 * h, 32), g, ds(mc * MC, MC)]
                            else:
                                lk = k3_sb[:, g, ds(nt * 128, w)]
                                rq = lambda mc: q3_sb[:, g, ds(mc * MC, MC)]
                            for mc in range(2):
                                nc.tensor.matmul(
                                    st[:w, mc, 0:MC],
                                    lhsT=lk,
                                    rhs=rq(mc),
                                    start=True,
                                    stop=True,
                                )
                            ex_out = ex_sb[0:w, slot, h, :].rearrange(
                                "p (a b) -> p a b", a=2
                            )
                            st_in = st[:w, :, 0:MC]
                            e = eng_of(codes[h]) if fastexp else "act"
                            if e == "act":
                                nc.scalar.activation(ex_out, st_in, AF.Exp)
                            else:
                                eng = nc.vector if e == "dve" else nc.gpsimd
                                eng.tensor_scalar(
                                    ex_out.bitcast(I16),
                                    st_in,
                                    EXP_A16,
                                    EXP_B16,
                                    op0=ALU.mult,
                                    op1=ALU.add,
                                )
                            del st
                    # interleave remaining QKV into early nts of group 0
                    if g == 0:
                        if nt == 1:
                            emit_k_pair(0, 1, nc.vector)
                        if nt == 2:
                            emit_k_pair(1, 1, nc.vector)
                        if nt == 3:
                            emit_vt_pair(2, nc.gpsimd)
                    if nt >= 1 and nt % 2 == 0 and nt + 2 < 14:
                        if g == 0 or nt + 2 > 4:
                            emit_vt_pair(nt + 2, nc.gpsimd)
                    # A@V one nt behind
                    if nt >= 1:
                        pw = NT_SIZES[nt - 1]
                        pslot = (nt - 1) % 3
                        for j in range(2):
                            for hh in range(2):
                                h = 2 * j + hh
                                for mt in range(7):
                                    mw = MT_SIZES[mt]
                                    nc.tensor.matmul(
                                        ot_ps[j][:mw, hh, mt, 0:33],
                                        lhsT=ex_sb[0:pw, pslot, h, ds(mt * 128, mw)],
                                        rhs=vt_sb[0:pw, nt - 1, 4 * g + h, :],
                                        start=(nt == 1),
                                        stop=(nt == 14),
                                    )

                # ---- normalize O^T, DMA-transpose to O ----
                rs = []
                for j in range(2):
                    r = sb.tile([128, 2, 7], F32, tag=f"rs{j}", name=f"rs{g}{j}")
                    nc.vector.reciprocal(r, ot_ps[j][:, :, :, 32:33])
                    rs.append(r)
                # keep the PE p-state warm through the normalize bubble
                if g == 1:
                    warm_pe(8)
                for mt in range(7):
                    mw = MT_SIZES[mt]
                    for j in range(2):
                        for hh in range(2):
                            hg = 4 * g + 2 * j + hh
                            eng = nc.vector if (mt % 2 == 0) else nc.gpsimd
                            eng.tensor_scalar(
                                on_t[:mw, mt, ds(32 * hg, 32)],
                                ot_ps[j][:mw, hh, mt, 0:32],
                                rs[j][:mw, hh, mt : mt + 1],
                                None,
                                op0=ALU.mult,
                            )
                    tp = st_tile()
                    nc.tensor.transpose(
                        tp[:, 0, 0:mw],
                        on_t[:mw, mt, ds(128 * g, 128)],
                        ident[:mw, :mw],
                    )
                    eng = nc.vector if (mt % 2 == 0) else nc.gpsimd
                    eng.tensor_copy(on_sb[:, g, ds(mt * 128, mw)], tp[:, 0, 0:mw])
                    del tp
                del ot_ps

            # ---- projection + bias (ACT idle at tail) ----
            for mc in range(2):
                for ot in range(2):
                    yp = st_tile()
                    for g2 in range(2):
                        nc.tensor.matmul(
                            yp[:, 0, 0:MC],
                            lhsT=wp_sb[:, g2, ot, :],
                            rhs=on_sb[:, g2, ds(mc * MC, MC)],
                            start=(g2 == 0),
                            stop=(g2 == 1),
                        )
                    nc.scalar.activation(
                        y_sb[:, ot, ds(mc * MC, MC)], yp[:, 0, 0:MC], AF.Identity,
                        bias=b_sb[:, ot, :], scale=1.0,
                    )
                    nc.sync.dma_start(
                        out=y_d.rearrange("(ot p) m -> ot p m", p=128)[ot, :, ds(mc * MC, MC)],
                        in_=y_sb[:, ot, ds(mc * MC, MC)],
                    )
                    del yp

    nc.compile()
    return nc


_NC = None


def _get_nc():
    global _NC
    if _NC is None:
        _NC = build_program()
    return _NC


def _bf16(a):
    return a.astype(mybir.dt.np(BF16))


def make_in_maps(x, w_qkv, w_proj, b_proj):
    x = np.asarray(x, np.float32)
    w_qkv = np.asarray(w_qkv, np.float32)
    w_proj = np.asarray(w_proj, np.float32)
    b_proj = np.asarray(b_proj, np.float32)
    P = x.shape[0]
    xf = np.ascontiguousarray(x.reshape(P, C, N))
    wqT = _bf16(np.ascontiguousarray((w_qkv[0:C] * SCALE).T))
    wkT = _bf16(np.ascontiguousarray(w_qkv[C : 2 * C].T))
    wvT = _bf16(np.ascontiguousarray(w_qkv[2 * C : 3 * C].T))
    wpT = _bf16(np.ascontiguousarray(w_proj.T))
    bias = np.ascontiguousarray(b_proj.reshape(C, 1))
    in_maps = []
    for core in range(N_CORES):
        p, mh = divmod(core, 2)
        if mh == 0:
            xp = xf[p]
        else:
            # rotate the n axis so this core's query half comes first
            xp = np.concatenate([xf[p][:, M:], xf[p][:, :M]], axis=1)
        in_maps.append(
            {
                "xf": _bf16(np.ascontiguousarray(xp)),
                "wqT": wqT,
                "wkT": wkT,
                "wvT": wvT,
                "wpT": wpT,
                "bias": bias,
            }
        )
    return in_maps


def assemble_output(per_core_y, x_shape):
    P, B, _, H, W, D = x_shape
    y = np.empty((P, C, N), np.float32)
    for core in range(N_CORES):
        p, mh = divmod(core, 2)
        y[p][:, mh * M : (mh + 1) * M] = per_core_y[core]
    return y.reshape(P, B, C, H, W, D)


def kernel(x, w_qkv, w_proj, b_proj):
    nc = _get_nc()
    in_maps = make_in_maps(x, w_qkv, w_proj, b_proj)
    res = run_bass_kernel_spmd(nc, in_maps, core_ids=list(range(N_CORES)))
    return assemble_output([res.results[c]["y"] for c in range(N_CORES)], x.shape)


# revision 14
# speedup vs baseline: 1.0686x; 1.0686x over previous
"""Trainium2 Bass kernel for nn_Attention_p_2757369004155.

Reference math (per p in 0..4):
  x [256, 1728] -> qkv = W_qkv @ x -> 8 heads of dim 32, N=1728
  attn = softmax((q*scale)^T k), out = v @ attn^T, y = W_p @ out + b

Sharding: 8 cores = 4 p-branches x 2 query-halves. Each core is fully
self-contained (K/V computed for all n, Q for its half). The host permutes
each core's n axis so its query half is always columns [0, 864).

v2 design (cost-model-driven):
  - All matmul MOVING operands are bf16 (1 cyc/row vs fp32's 4). Stationary
    loads are free in the cost model, so A@V uses exp(S^T) tiles as the
    stationary operand and streams only 33 columns (32 v-dims + ones column
    for the softmax denominator) per (head, m-tile).
  - S^T[n, m] = K^T Q per head as single 32-deep bf16 matmuls, sliced
    directly out of the K/Q SBUF tiles at partition offset 32h (no
    stacking DMAs, no f32r split).
  - ACT (scalar engine) does ONLY exp: one 864-col call per (head, nt)
    psum tile. This is the roofline engine (~12M exps/core).
  - A tunable share of exp tiles is offloaded to DVE/Pool via a one-op
    Schraudolph fast-exp: int16(A*x + B) bit-cast to bf16. |rel err| ~3%
    sawtooth on those tiles only; softmax-consistent (denominator uses the
    same approximated weights), so end-to-end error stays ~1e-3..1e-2.
  - PSUM: 3 rotating [128, 2, 512] S^T tiles (6 banks) + 2 single-bank
    A@V accumulators [128, 2, 7, 33]. QKV/proj psum reuses the S^T tag.
  - O^T -> O via DMA transpose (SP queue, async) instead of PE transposes.
  - Output projection + bias on ACT at the tail when it is idle.
"""

import numpy as np

import concourse.bass as bass
import concourse.tile as tile
from concourse import bacc, mybir
from concourse.bass import ds
from concourse.bass_utils import run_bass_kernel_spmd
from concourse.masks import make_identity

F32 = mybir.dt.float32
BF16 = mybir.dt.bfloat16
I16 = mybir.dt.int16
AF = mybir.ActivationFunctionType
ALU = mybir.AluOpType

N_CORES = 8
C = 256            # channels
NH = 8             # heads
HD = 32            # head dim
N = 1728           # sequence (12*12*12)
M = N // 2         # per-core query positions
MC = 432           # m chunk (psum bank = 512 fp32)
NT_SIZES = [128] * 13 + [64]          # n contraction tiles
MT_SIZES = [128] * 6 + [96]           # m tiles for the A@V stationary
SCALE = HD ** -0.5

# Schraudolph one-op fast-exp constants for bf16 bit-trick:
#   bf16_bits(exp(x)) ~= int16(A16 * x + B16)
# A16 = 2^7 / ln 2; B16 tuned numerically for min-max relative error on
# x in [-0.8, 0.8] assuming round-to-nearest on the fp32->int16 convert.
EXP_A16 = 184.6650390625
EXP_B16 = 16250.25

# exp work assignment: for each (g, nt) the four heads' tiles go to these
# engines. "a"=ACT exact exp, "d"=DVE fast-exp, "p"=Pool fast-exp.
# Tuned against TimelineSim.
ASSIGN_DEFAULT = "aaaa"


def build_program(assign=None, fastexp=True):
    nc = bacc.Bacc(
        "TRN2",
        target_bir_lowering=False,
        debug=False,
        enable_asserts=False,
        num_devices=N_CORES,
    )

    xf_d = nc.dram_tensor("xf", [C, N], BF16, kind="ExternalInput").ap()
    wq_d = nc.dram_tensor("wqT", [C, C], BF16, kind="ExternalInput").ap()
    wk_d = nc.dram_tensor("wkT", [C, C], BF16, kind="ExternalInput").ap()
    wv_d = nc.dram_tensor("wvT", [C, C], BF16, kind="ExternalInput").ap()
    wp_d = nc.dram_tensor("wpT", [C, C], BF16, kind="ExternalInput").ap()
    b_d = nc.dram_tensor("bias", [C, 1], F32, kind="ExternalInput").ap()
    y_d = nc.dram_tensor("y", [C, M], F32, kind="ExternalOutput").ap()

    xf_r = xf_d.rearrange("(kt p) n -> p kt n", p=128)

    if assign is None:
        assign = {}

    def eng_of(code):
        return {"a": "act", "d": "dve", "p": "pool"}[code]

    with tile.TileContext(nc) as tc:
        with (
            tc.tile_pool(name="persist", bufs=1) as sb,
            tc.tile_pool(name="ps", bufs=1, space="PSUM") as ps,
        ):
            # ---- input DMAs (SP queue; xf chunk 0 + wq first so Q starts early) ----
            wq_sb = sb.tile([128, 2, 2, 128], BF16, tag="wq")
            wk_sb = sb.tile([128, 2, 2, 128], BF16, tag="wk")
            xf_sb = sb.tile([128, 2, N], BF16, tag="xf")
            nc.sync.dma_start(out=wq_sb, in_=wq_d.rearrange("(kt p) (ot o) -> p kt ot o", p=128, o=128))
            nc.sync.dma_start(out=xf_sb[:, :, ds(0, MC)], in_=xf_r[:, :, ds(0, MC)])
            nc.sync.dma_start(out=wk_sb, in_=wk_d.rearrange("(kt p) (ot o) -> p kt ot o", p=128, o=128))
            nc.sync.dma_start(out=xf_sb[:, :, ds(MC, MC)], in_=xf_r[:, :, ds(MC, MC)])
            wv_sb = sb.tile([128, 2, 256], BF16, tag="wv")
            nc.sync.dma_start(out=wv_sb, in_=wv_d.rearrange("(kt p) c -> p kt c", p=128))
            for ch in range(2, 4):
                nc.sync.dma_start(out=xf_sb[:, :, ds(ch * MC, MC)], in_=xf_r[:, :, ds(ch * MC, MC)])
            wp_sb = sb.tile([128, 2, 2, 128], BF16, tag="wp")
            b_sb = sb.tile([128, 2, 1], F32, tag="b")
            nc.gpsimd.dma_start(out=wp_sb, in_=wp_d.rearrange("(kt p) (ot o) -> p kt ot o", p=128, o=128))
            nc.gpsimd.dma_start(out=b_sb, in_=b_d.rearrange("(ot p) one -> p ot one", p=128))

            # trigger the ACT exp table load during the initial DMA window
            warm = sb.tile([128, 1], F32, tag="warm")
            nc.vector.memset(warm, 0.0)
            nc.scalar.activation(warm, warm, AF.Exp)
            ident = sb.tile([128, 128], F32, tag="ident")
            make_identity(nc, ident)

            # ---- persistent SBUF ----
            k_sb = sb.tile([128, 2, N], BF16, tag="k")       # [32h+d, g, n]
            q_sb = sb.tile([128, 2, M], BF16, tag="q")       # [32h+d, g, m]
            # head 3 lives at partition base 96, which matmul APs cannot
            # address (allowed bases: 0/32/64) — DMA-shift it to base 0
            k3_sb = sb.tile([32, 2, N], BF16, tag="k3")
            q3_sb = sb.tile([32, 2, M], BF16, tag="q3")
            vt_sb = sb.tile([128, 14, NH, 33], BF16, tag="vt")  # [n, nt, head, c+1]
            nc.vector.memset(vt_sb[:, :, :, 32:33], 1.0)
            ex_sb = sb.tile([128, 4, 4, M], BF16, tag="ex")  # [n, slot, h-of-g, m]
            on_t = sb.tile([128, 7, C], F32, tag="on_t")     # [m, mt, c]
            on_sb = sb.tile([128, 2, M], BF16, tag="on")     # [c-half, g2, m]
            y_sb = sb.tile([128, 2, M], F32, tag="y")
            wu_in = sb.tile([128, 128], BF16, tag="wu_in")
            nc.vector.memset(wu_in, 0.0)

            def st_tile():
                return ps.tile([128, 2, 512], F32, tag="st", name="st", bufs=3)

            def warm_pe(n_mm, cols=512):
                for _ in range(n_mm):
                    wu = st_tile()
                    nc.tensor.matmul(
                        wu[:, 0, 0:128], lhsT=wu_in, rhs=wu_in,
                        start=True, stop=True,
                    )
                    del wu

            # PE p-state warmup while input DMAs land
            warm_pe(9)

            # ---- Q projections (both groups; g1 needed a bit later) ----
            def emit_q(g):
                pq = st_tile()
                for mc in range(2):
                    for kt in range(2):
                        nc.tensor.matmul(
                            pq[:, mc, 0:MC],
                            lhsT=wq_sb[:, kt, g, :],
                            rhs=xf_sb[:, kt, ds(mc * MC, MC)],
                            start=(kt == 0),
                            stop=(kt == 1),
                        )
                nc.vector.tensor_copy(
                    q_sb[:, g, :].rearrange("p (a b) -> p a b", a=2),
                    pq[:, :, 0:MC],
                )
                nc.sync.dma_start(out=q3_sb[:, g, :], in_=q_sb[ds(96, 32), g, :])
                del pq

            def emit_k_pair(g, half, engine):
                """K for n-range [864*half, 864*half+864), one head group."""
                pk = st_tile()
                for mc in range(2):
                    for kt in range(2):
                        nc.tensor.matmul(
                            pk[:, mc, 0:MC],
                            lhsT=wk_sb[:, kt, g, :],
                            rhs=xf_sb[:, kt, ds(half * M + mc * MC, MC)],
                            start=(kt == 0),
                            stop=(kt == 1),
                        )
                engine.tensor_copy(
                    k_sb[:, g, ds(half * M, M)].rearrange("p (a b) -> p a b", a=2),
                    pk[:, :, 0:MC],
                )
                nc.sync.dma_start(
                    out=k3_sb[:, g, ds(half * M, M)],
                    in_=k_sb[ds(96, 32), g, ds(half * M, M)],
                )

            def emit_vt_pair(nt, engine):
                """V^T tiles for nt and nt+1 (both head groups) in one st tile."""
                pv = st_tile()
                sizes = [NT_SIZES[nt], NT_SIZES[nt + 1] if nt + 1 < 14 else 0]
                for i, w in enumerate(sizes):
                    if w == 0:
                        continue
                    for g in range(2):
                        for kt in range(2):
                            nc.tensor.matmul(
                                pv[:w, i, ds(128 * g, 128)],
                                lhsT=xf_sb[:, kt, ds((nt + i) * 128, w)],
                                rhs=wv_sb[:, kt, ds(128 * g, 128)],
                                start=(kt == 0),
                                stop=(kt == 1),
                            )
                for i, w in enumerate(sizes):
                    if w == 0:
                        continue
                    engine.tensor_copy(
                        vt_sb[:w, nt + i, :, 0:32],
                        pv[:w, i, 0:256].rearrange("p (h c) -> p h c", h=8),
                    )

            emit_q(0)
            emit_q(1)

            # ---- attention schedule ----
            # g0: A@V lag 1; g1: lag 2 (so g0's epilogue can drain the ot
            # accumulators before g1's first A@V reuses them).
            ex_out_of = None  # set below

            def emit_st_exp(g, nt, codes):
                w = NT_SIZES[nt]
                slot = nt % 4
                for h in range(4):
                    st = st_tile()
                    if h < 3:
                        lk = k_sb[ds(32 * h, 32), g, ds(nt * 128, w)]
                        rq0 = q_sb[ds(32 * h, 32), g, ds(0, MC)]
                        rq1 = q_sb[ds(32 * h, 32), g, ds(MC, MC)]
                    else:
                        lk = k3_sb[:, g, ds(nt * 128, w)]
                        rq0 = q3_sb[:, g, ds(0, MC)]
                        rq1 = q3_sb[:, g, ds(MC, MC)]
                    for mc, rq in ((0, rq0), (1, rq1)):
                        nc.tensor.matmul(
                            st[:w, mc, 0:MC], lhsT=lk, rhs=rq,
                            start=True, stop=True,
                        )
                    ex_out = ex_sb[0:w, slot, h, :].rearrange("p (a b) -> p a b", a=2)
                    st_in = st[:w, :, 0:MC]
                    e = eng_of(codes[h]) if fastexp else "act"
                    if e == "act":
                        nc.scalar.activation(ex_out, st_in, AF.Exp)
                    else:
                        eng = nc.vector if e == "dve" else nc.gpsimd
                        eng.tensor_scalar(
                            ex_out.bitcast(I16), st_in,
                            EXP_A16, EXP_B16,
                            op0=ALU.mult, op1=ALU.add,
                        )
                    del st

            def emit_av(g, pnt, ot_ps, first, last):
                pw = NT_SIZES[pnt]
                pslot = pnt % 4
                for j in range(2):
                    for hh in range(2):
                        h = 2 * j + hh
                        for mt in range(7):
                            mw = MT_SIZES[mt]
                            nc.tensor.matmul(
                                ot_ps[j][:mw, hh, mt, 0:33],
                                lhsT=ex_sb[0:pw, pslot, h, ds(mt * 128, mw)],
                                rhs=vt_sb[0:pw, pnt, 4 * g + h, :],
                                start=first, stop=last,
                            )

            def emit_recips(g, ot_ps):
                rs = []
                for j in range(2):
                    r = sb.tile([128, 2, 7], F32, tag=f"rs{j}", name=f"rs{g}{j}")
                    nc.vector.reciprocal(r, ot_ps[j][:, :, :, 32:33])
                    rs.append(r)
                return rs

            def emit_normalize(g, ot_ps, rs):
                # all 14 ops up-front, split across DVE/Pool, so the ot
                # accumulators are released as fast as possible
                for mt in range(7):
                    mw = MT_SIZES[mt]
                    for j in range(2):
                        for hh in range(2):
                            hg = 4 * g + 2 * j + hh
                            eng = nc.vector if ((mt + hh) % 2 == 0) else nc.gpsimd
                            eng.tensor_scalar(
                                on_t[:mw, mt, ds(32 * hg, 32)],
                                ot_ps[j][:mw, hh, mt, 0:32],
                                rs[j][:mw, hh, mt : mt + 1],
                                None,
                                op0=ALU.mult,
                            )

            def emit_transpose(g, mt):
                mw = MT_SIZES[mt]
                tp = st_tile()
                nc.tensor.transpose(
                    tp[:, 0, 0:mw],
                    on_t[:mw, mt, ds(128 * g, 128)],
                    ident[:mw, :mw],
                )
                eng = nc.vector if (mt % 2 == 0) else nc.gpsimd
                eng.tensor_copy(on_sb[:, g, ds(mt * 128, mw)], tp[:, 0, 0:mw])
                del tp

            def emit_proj(mc, ot, bias_eng):
                yp = st_tile()
                for g2 in range(2):
                    nc.tensor.matmul(
                        yp[:, 0, 0:MC],
                        lhsT=wp_sb[:, g2, ot, :],
                        rhs=on_sb[:, g2, ds(mc * MC, MC)],
                        start=(g2 == 0), stop=(g2 == 1),
                    )
                if bias_eng == "act":
                    nc.scalar.activation(
                        y_sb[:, ot, ds(mc * MC, MC)], yp[:, 0, 0:MC], AF.Identity,
                        bias=b_sb[:, ot, :], scale=1.0,
                    )
                else:
                    nc.vector.tensor_scalar(
                        y_sb[:, ot, ds(mc * MC, MC)], yp[:, 0, 0:MC],
                        b_sb[:, ot, :], None, op0=ALU.add,
                    )
                nc.sync.dma_start(
                    out=y_d.rearrange("(ot p) m -> ot p m", p=128)[ot, :, ds(mc * MC, MC)],
                    in_=y_sb[:, ot, ds(mc * MC, MC)],
                )
                del yp

            LAG = {0: 1, 1: 2}
            ot_live = None
            rs_prev = None

            for g in range(2):
                lag = LAG[g]
                ot_ps = [
                    ps.tile([128, 2, 7, 33], F32, tag=f"ot{j}", name=f"ot{g}{j}", bufs=1)
                    for j in range(2)
                ]
                for nt in range(14 + lag):
                    if nt < 14:
                        codes = assign.get((g, nt), ASSIGN_DEFAULT)
                        emit_st_exp(g, nt, codes)
                    # fillers: group 0 interleaves the rest of QKV; group 1
                    # interleaves group 0's epilogue transposes
                    if g == 0:
                        if nt == 0:
                            emit_k_pair(0, 1, nc.vector)
                            emit_vt_pair(0, nc.gpsimd)
                        elif nt == 1:
                            emit_k_pair(1, 1, nc.vector)
                            emit_vt_pair(2, nc.gpsimd)
                        elif 2 <= nt <= 6:
                            emit_vt_pair(2 * nt, nc.gpsimd)
                    else:
                        if nt == 0:
                            rs_prev = emit_recips(0, ot_live)
                            emit_normalize(0, ot_live, rs_prev)
                        elif nt in (1, 2, 3):
                            for mt in range(2 * (nt - 1), min(2 * nt, 7)):
                                emit_transpose(0, mt)
                            if nt == 3:
                                emit_transpose(0, 6)
                                del ot_live
                    if nt >= lag and nt - lag < 14:
                        emit_av(g, nt - lag, ot_ps, first=(nt == lag), last=(nt - lag == 13))
                ot_live = ot_ps

            # ---- tail: normalize g1, transpose, project, bias, store ----
            rs1 = emit_recips(1, ot_live)
            warm_pe(3)
            emit_normalize(1, ot_live, rs1)
            for mt in range(4):
                emit_transpose(1, mt)
            for ot in range(2):
                emit_proj(0, ot, "act" if ot == 0 else "dve")
            for mt in range(4, 7):
                emit_transpose(1, mt)
            for ot in range(2):
                emit_proj(1, ot, "act" if ot == 0 else "dve")

    nc.compile()
    return nc


_NC = None


def _get_nc():
    global _NC
    if _NC is None:
        _NC = build_program()
    return _NC


def _bf16(a):
    return a.astype(mybir.dt.np(BF16))


def make_in_maps(x, w_qkv, w_proj, b_proj):
    x = np.asarray(x, np.float32)
    w_qkv = np.asarray(w_qkv, np.float32)
    w_proj = np.asarray(w_proj, np.float32)
    b_proj = np.asarray(b_proj, np.float32)
    P = x.shape[0]
    xf = np.ascontiguousarray(x.reshape(P, C, N))
    wqT = _bf16(np.ascontiguousarray((w_qkv[0:C] * SCALE).T))
    wkT = _bf16(np.ascontiguousarray(w_qkv[C : 2 * C].T))
    wvT = _bf16(np.ascontiguousarray(w_qkv[2 * C : 3 * C].T))
    wpT = _bf16(np.ascontiguousarray(w_proj.T))
    bias = np.ascontiguousarray(b_proj.reshape(C, 1))
    in_maps = []
    for core in range(N_CORES):
        p, mh = divmod(core, 2)
        if mh == 0:
            xp = xf[p]
        else:
            # rotate the n axis so this core's query half comes first
            xp = np.concatenate([xf[p][:, M:], xf[p][:, :M]], axis=1)
        in_maps.append(
            {
                "xf": _bf16(np.ascontiguousarray(xp)),
                "wqT": wqT,
                "wkT": wkT,
                "wvT": wvT,
                "wpT": wpT,
                "bias": bias,
            }
        )
    return in_maps


def assemble_output(per_core_y, x_shape):
    P, B, _, H, W, D = x_shape
    y = np.empty((P, C, N), np.float32)
    for core in range(N_CORES):
        p, mh = divmod(core, 2)
        y[p][:, mh * M : (mh + 1) * M] = per_core_y[core]
    return y.reshape(P, B, C, H, W, D)


def kernel(x, w_qkv, w_proj, b_proj):
    nc = _get_nc()
    in_maps = make_in_maps(x, w_qkv, w_proj, b_proj)
    res = run_bass_kernel_spmd(nc, in_maps, core_ids=list(range(N_CORES)))
    return assemble_output([res.results[c]["y"] for c in range(N_CORES)], x.shape)


# revision 15
# speedup vs baseline: 1.3212x; 1.2364x over previous
"""Trainium2 Bass kernel for nn_Attention_p_2757369004155.

Reference math (per p in 0..4):
  x [256, 1728] -> qkv = W_qkv @ x -> 8 heads of dim 32, N=1728
  attn = softmax((q*scale)^T k), out = v @ attn^T, y = W_p @ out + b

Sharding: 8 cores = 4 p-branches x 2 query-halves. Each core is fully
self-contained (K/V computed for all n, Q for its half). The host permutes
each core's n axis so its query half is always columns [0, 864).

v2 design (cost-model-driven):
  - All matmul MOVING operands are bf16 (1 cyc/row vs fp32's 4). Stationary
    loads are free in the cost model, so A@V uses exp(S^T) tiles as the
    stationary operand and streams only 33 columns (32 v-dims + ones column
    for the softmax denominator) per (head, m-tile).
  - S^T[n, m] = K^T Q per head as single 32-deep bf16 matmuls, sliced
    directly out of the K/Q SBUF tiles at partition offset 32h (no
    stacking DMAs, no f32r split).
  - ACT (scalar engine) does ONLY exp: one 864-col call per (head, nt)
    psum tile. This is the roofline engine (~12M exps/core).
  - A tunable share of exp tiles is offloaded to DVE/Pool via a one-op
    Schraudolph fast-exp: int16(A*x + B) bit-cast to bf16. |rel err| ~3%
    sawtooth on those tiles only; softmax-consistent (denominator uses the
    same approximated weights), so end-to-end error stays ~1e-3..1e-2.
  - PSUM: 3 rotating [128, 2, 512] S^T tiles (6 banks) + 2 single-bank
    A@V accumulators [128, 2, 7, 33]. QKV/proj psum reuses the S^T tag.
  - O^T -> O via DMA transpose (SP queue, async) instead of PE transposes.
  - Output projection + bias on ACT at the tail when it is idle.
"""

import numpy as np

import concourse.bass as bass
import concourse.tile as tile
from concourse import bacc, mybir
from concourse.bass import ds
from concourse.bass_utils import run_bass_kernel_spmd
from concourse.masks import make_identity

F32 = mybir.dt.float32
BF16 = mybir.dt.bfloat16
I16 = mybir.dt.int16
AF = mybir.ActivationFunctionType
ALU = mybir.AluOpType

N_CORES = 8
C = 256            # channels
NH = 8             # heads
HD = 32            # head dim
N = 1728           # sequence (12*12*12)
M = N // 2         # per-core query positions
MC = 432           # m chunk (psum bank = 512 fp32)
NT_SIZES = [128] * 13 + [64]          # n contraction tiles
MT_SIZES = [128] * 6 + [96]           # m tiles for the A@V stationary
SCALE = HD ** -0.5

# Schraudolph one-op fast-exp constants for bf16 bit-trick:
#   bf16_bits(exp(x)) ~= int16(A16 * x + B16)
# A16 = 2^7 / ln 2; B16 tuned numerically for min-max relative error on
# x in [-0.8, 0.8] assuming round-to-nearest on the fp32->int16 convert.
EXP_A16 = 184.6650390625
EXP_B16 = 16250.25

# exp work assignment: for each (g, nt) the four heads' tiles go to these
# engines. "a"=ACT exact exp, "d"=DVE fast-exp, "p"=Pool fast-exp.
# Tuned against TimelineSim.
ASSIGN_DEFAULT = "aaaa"


def build_program(assign=None, fastexp=True):
    nc = bacc.Bacc(
        "TRN2",
        target_bir_lowering=False,
        debug=False,
        enable_asserts=False,
        num_devices=N_CORES,
    )

    xf_d = nc.dram_tensor("xf", [C, N], BF16, kind="ExternalInput").ap()
    wq_d = nc.dram_tensor("wqT", [C, C], BF16, kind="ExternalInput").ap()
    wk_d = nc.dram_tensor("wkT", [C, C], BF16, kind="ExternalInput").ap()
    wv_d = nc.dram_tensor("wvT", [C, C], BF16, kind="ExternalInput").ap()
    wp_d = nc.dram_tensor("wpT", [C, C], BF16, kind="ExternalInput").ap()
    b_d = nc.dram_tensor("bias", [C, 1], F32, kind="ExternalInput").ap()
    y_d = nc.dram_tensor("y", [C, M], F32, kind="ExternalOutput").ap()

    xf_r = xf_d.rearrange("(kt p) n -> p kt n", p=128)

    if assign is None:
        assign = {}
        if fastexp:
            cycle = ["aadp", "adpa", "dpaa", "apda"]
            i = 0
            for g_ in range(2):
                for nt_ in range(14):
                    assign[(g_, nt_)] = cycle[i % len(cycle)]
                    i += 1

    def eng_of(code):
        return {"a": "act", "d": "dve", "p": "pool"}[code]

    with tile.TileContext(nc) as tc:
        with (
            tc.tile_pool(name="persist", bufs=1) as sb,
            tc.tile_pool(name="ps", bufs=1, space="PSUM") as ps,
        ):
            # ---- input DMAs (SP queue; xf chunk 0 + wq first so Q starts early) ----
            wq_sb = sb.tile([128, 2, 2, 128], BF16, tag="wq")
            wk_sb = sb.tile([128, 2, 2, 128], BF16, tag="wk")
            xf_sb = sb.tile([128, 2, N], BF16, tag="xf")
            nc.sync.dma_start(out=wq_sb, in_=wq_d.rearrange("(kt p) (ot o) -> p kt ot o", p=128, o=128))
            nc.sync.dma_start(out=xf_sb[:, :, ds(0, MC)], in_=xf_r[:, :, ds(0, MC)])
            nc.sync.dma_start(out=wk_sb, in_=wk_d.rearrange("(kt p) (ot o) -> p kt ot o", p=128, o=128))
            nc.sync.dma_start(out=xf_sb[:, :, ds(MC, MC)], in_=xf_r[:, :, ds(MC, MC)])
            wv_sb = sb.tile([128, 2, 256], BF16, tag="wv")
            nc.sync.dma_start(out=wv_sb, in_=wv_d.rearrange("(kt p) c -> p kt c", p=128))
            for ch in range(2, 4):
                nc.sync.dma_start(out=xf_sb[:, :, ds(ch * MC, MC)], in_=xf_r[:, :, ds(ch * MC, MC)])
            wp_sb = sb.tile([128, 2, 2, 128], BF16, tag="wp")
            b_sb = sb.tile([128, 2, 1], F32, tag="b")
            nc.gpsimd.dma_start(out=wp_sb, in_=wp_d.rearrange("(kt p) (ot o) -> p kt ot o", p=128, o=128))
            nc.gpsimd.dma_start(out=b_sb, in_=b_d.rearrange("(ot p) one -> p ot one", p=128))

            # trigger the ACT exp table load during the initial DMA window
            warm = sb.tile([128, 1], F32, tag="warm")
            nc.vector.memset(warm, 0.0)
            nc.scalar.activation(warm, warm, AF.Exp)
            ident = sb.tile([128, 128], F32, tag="ident")
            make_identity(nc, ident)

            # ---- persistent SBUF ----
            k_sb = sb.tile([128, 2, N], BF16, tag="k")       # [32h+d, g, n]
            q_sb = sb.tile([128, 2, M], BF16, tag="q")       # [32h+d, g, m]
            # head 3 lives at partition base 96, which matmul APs cannot
            # address (allowed bases: 0/32/64) — DMA-shift it to base 0
            k3_sb = sb.tile([32, 2, N], BF16, tag="k3")
            q3_sb = sb.tile([32, 2, M], BF16, tag="q3")
            vt_sb = sb.tile([128, 14, NH, 33], BF16, tag="vt")  # [n, nt, head, c+1]
            nc.vector.memset(vt_sb[:, :, :, 32:33], 1.0)
            ex_sb = sb.tile([128, 4, 4, M], BF16, tag="ex")  # [n, slot, h-of-g, m]
            on_t = sb.tile([128, 7, C], F32, tag="on_t")     # [m, mt, c]
            on_sb = sb.tile([128, 2, M], BF16, tag="on")     # [c-half, g2, m]
            y_sb = sb.tile([128, 2, M], F32, tag="y")
            wu_in = sb.tile([128, 128], BF16, tag="wu_in")
            nc.vector.memset(wu_in, 0.0)

            def st_tile():
                return ps.tile([128, 2, 512], F32, tag="st", name="st", bufs=3)

            def warm_pe(n_mm, cols=512):
                for _ in range(n_mm):
                    wu = st_tile()
                    nc.tensor.matmul(
                        wu[:, 0, 0:128], lhsT=wu_in, rhs=wu_in,
                        start=True, stop=True,
                    )
                    del wu

            # PE p-state warmup while input DMAs land
            warm_pe(9)

            # ---- Q projections (both groups; g1 needed a bit later) ----
            def emit_q(g):
                pq = st_tile()
                for mc in range(2):
                    for kt in range(2):
                        nc.tensor.matmul(
                            pq[:, mc, 0:MC],
                            lhsT=wq_sb[:, kt, g, :],
                            rhs=xf_sb[:, kt, ds(mc * MC, MC)],
                            start=(kt == 0),
                            stop=(kt == 1),
                        )
                nc.vector.tensor_copy(
                    q_sb[:, g, :].rearrange("p (a b) -> p a b", a=2),
                    pq[:, :, 0:MC],
                )
                nc.sync.dma_start(out=q3_sb[:, g, :], in_=q_sb[ds(96, 32), g, :])
                del pq

            def emit_k_pair(g, half, engine):
                """K for n-range [864*half, 864*half+864), one head group."""
                pk = st_tile()
                for mc in range(2):
                    for kt in range(2):
                        nc.tensor.matmul(
                            pk[:, mc, 0:MC],
                            lhsT=wk_sb[:, kt, g, :],
                            rhs=xf_sb[:, kt, ds(half * M + mc * MC, MC)],
                            start=(kt == 0),
                            stop=(kt == 1),
                        )
                engine.tensor_copy(
                    k_sb[:, g, ds(half * M, M)].rearrange("p (a b) -> p a b", a=2),
                    pk[:, :, 0:MC],
                )
                nc.sync.dma_start(
                    out=k3_sb[:, g, ds(half * M, M)],
                    in_=k_sb[ds(96, 32), g, ds(half * M, M)],
                )

            def emit_vt_pair(nt, engine):
                """V^T tiles for nt and nt+1 (both head groups) in one st tile."""
                pv = st_tile()
                sizes = [NT_SIZES[nt], NT_SIZES[nt + 1] if nt + 1 < 14 else 0]
                for i, w in enumerate(sizes):
                    if w == 0:
                        continue
                    for g in range(2):
                        for kt in range(2):
                            nc.tensor.matmul(
                                pv[:w, i, ds(128 * g, 128)],
                                lhsT=xf_sb[:, kt, ds((nt + i) * 128, w)],
                                rhs=wv_sb[:, kt, ds(128 * g, 128)],
                                start=(kt == 0),
                                stop=(kt == 1),
                            )
                for i, w in enumerate(sizes):
                    if w == 0:
                        continue
                    engine.tensor_copy(
                        vt_sb[:w, nt + i, :, 0:32],
                        pv[:w, i, 0:256].rearrange("p (h c) -> p h c", h=8),
                    )

            emit_q(0)
            emit_q(1)

            # ---- attention schedule ----
            # g0: A@V lag 1; g1: lag 2 (so g0's epilogue can drain the ot
            # accumulators before g1's first A@V reuses them).
            ex_out_of = None  # set below

            def emit_st_exp(g, nt, codes):
                w = NT_SIZES[nt]
                slot = nt % 4
                for h in range(4):
                    st = st_tile()
                    if h < 3:
                        lk = k_sb[ds(32 * h, 32), g, ds(nt * 128, w)]
                        rq0 = q_sb[ds(32 * h, 32), g, ds(0, MC)]
                        rq1 = q_sb[ds(32 * h, 32), g, ds(MC, MC)]
                    else:
                        lk = k3_sb[:, g, ds(nt * 128, w)]
                        rq0 = q3_sb[:, g, ds(0, MC)]
                        rq1 = q3_sb[:, g, ds(MC, MC)]
                    for mc, rq in ((0, rq0), (1, rq1)):
                        nc.tensor.matmul(
                            st[:w, mc, 0:MC], lhsT=lk, rhs=rq,
                            start=True, stop=True,
                        )
                    ex_out = ex_sb[0:w, slot, h, :].rearrange("p (a b) -> p a b", a=2)
                    st_in = st[:w, :, 0:MC]
                    e = eng_of(codes[h]) if fastexp else "act"
                    if e == "act":
                        nc.scalar.activation(ex_out, st_in, AF.Exp)
                    else:
                        eng = nc.vector if e == "dve" else nc.gpsimd
                        eng.tensor_scalar(
                            ex_out.bitcast(I16), st_in,
                            EXP_A16, EXP_B16,
                            op0=ALU.mult, op1=ALU.add,
                        )
                    del st

            def emit_av(g, pnt, ot_ps, first, last):
                pw = NT_SIZES[pnt]
                pslot = pnt % 4
                for j in range(2):
                    for hh in range(2):
                        h = 2 * j + hh
                        for mt in range(7):
                            mw = MT_SIZES[mt]
                            nc.tensor.matmul(
                                ot_ps[j][:mw, hh, mt, 0:33],
                                lhsT=ex_sb[0:pw, pslot, h, ds(mt * 128, mw)],
                                rhs=vt_sb[0:pw, pnt, 4 * g + h, :],
                                start=first, stop=last,
                            )

            def emit_recips(g, ot_ps):
                rs = []
                for j in range(2):
                    r = sb.tile([128, 2, 7], F32, tag=f"rs{j}", name=f"rs{g}{j}")
                    nc.vector.reciprocal(r, ot_ps[j][:, :, :, 32:33])
                    rs.append(r)
                return rs

            def emit_normalize(g, ot_ps, rs):
                # all 14 ops up-front, split across DVE/Pool, so the ot
                # accumulators are released as fast as possible
                for mt in range(7):
                    mw = MT_SIZES[mt]
                    for j in range(2):
                        for hh in range(2):
                            hg = 4 * g + 2 * j + hh
                            eng = nc.vector if ((mt + hh) % 2 == 0) else nc.gpsimd
                            eng.tensor_scalar(
                                on_t[:mw, mt, ds(32 * hg, 32)],
                                ot_ps[j][:mw, hh, mt, 0:32],
                                rs[j][:mw, hh, mt : mt + 1],
                                None,
                                op0=ALU.mult,
                            )

            def emit_transpose(g, mt):
                mw = MT_SIZES[mt]
                tp = st_tile()
                nc.tensor.transpose(
                    tp[:, 0, 0:mw],
                    on_t[:mw, mt, ds(128 * g, 128)],
                    ident[:mw, :mw],
                )
                eng = nc.vector if (mt % 2 == 0) else nc.gpsimd
                eng.tensor_copy(on_sb[:, g, ds(mt * 128, mw)], tp[:, 0, 0:mw])
                del tp

            def emit_proj(mc, ot, bias_eng):
                yp = st_tile()
                for g2 in range(2):
                    nc.tensor.matmul(
                        yp[:, 0, 0:MC],
                        lhsT=wp_sb[:, g2, ot, :],
                        rhs=on_sb[:, g2, ds(mc * MC, MC)],
                        start=(g2 == 0), stop=(g2 == 1),
                    )
                if bias_eng == "act":
                    nc.scalar.activation(
                        y_sb[:, ot, ds(mc * MC, MC)], yp[:, 0, 0:MC], AF.Identity,
                        bias=b_sb[:, ot, :], scale=1.0,
                    )
                else:
                    nc.vector.tensor_scalar(
                        y_sb[:, ot, ds(mc * MC, MC)], yp[:, 0, 0:MC],
                        b_sb[:, ot, :], None, op0=ALU.add,
                    )
                nc.sync.dma_start(
                    out=y_d.rearrange("(ot p) m -> ot p m", p=128)[ot, :, ds(mc * MC, MC)],
                    in_=y_sb[:, ot, ds(mc * MC, MC)],
                )
                del yp

            LAG = {0: 1, 1: 2}
            ot_live = None
            rs_prev = None

            for g in range(2):
                lag = LAG[g]
                ot_ps = [
                    ps.tile([128, 2, 7, 33], F32, tag=f"ot{j}", name=f"ot{g}{j}", bufs=1)
                    for j in range(2)
                ]
                for nt in range(14 + lag):
                    if nt < 14:
                        codes = assign.get((g, nt), ASSIGN_DEFAULT)
                        emit_st_exp(g, nt, codes)
                    # fillers: group 0 interleaves the rest of QKV; group 1
                    # interleaves group 0's epilogue transposes
                    if g == 0:
                        if nt == 0:
                            emit_k_pair(0, 1, nc.vector)
                            emit_vt_pair(0, nc.gpsimd)
                        elif nt == 1:
                            emit_k_pair(1, 1, nc.vector)
                            emit_vt_pair(2, nc.gpsimd)
                        elif 2 <= nt <= 6:
                            emit_vt_pair(2 * nt, nc.gpsimd)
                    else:
                        if nt == 0:
                            rs_prev = emit_recips(0, ot_live)
                            emit_normalize(0, ot_live, rs_prev)
                        elif nt in (1, 2, 3):
                            for mt in range(2 * (nt - 1), min(2 * nt, 7)):
                                emit_transpose(0, mt)
                            if nt == 3:
                                emit_transpose(0, 6)
                                del ot_live
                    if nt >= lag and nt - lag < 14:
                        emit_av(g, nt - lag, ot_ps, first=(nt == lag), last=(nt - lag == 13))
                ot_live = ot_ps

            # ---- tail: normalize g1, transpose, project, bias, store ----
            rs1 = emit_recips(1, ot_live)
            warm_pe(3)
            emit_normalize(1, ot_live, rs1)
            for mt in range(4):
                emit_transpose(1, mt)
            for ot in range(2):
                emit_proj(0, ot, "act" if ot == 0 else "dve")
            for mt in range(4, 7):
                emit_transpose(1, mt)
            for ot in range(2):
                emit_proj(1, ot, "act" if ot == 0 else "dve")

    nc.compile()
    return nc


_NC = None


def _get_nc():
    global _NC
    if _NC is None:
        _NC = build_program()
    return _NC


def _bf16(a):
    return a.astype(mybir.dt.np(BF16))


def make_in_maps(x, w_qkv, w_proj, b_proj):
    x = np.asarray(x, np.float32)
    w_qkv = np.asarray(w_qkv, np.float32)
    w_proj = np.asarray(w_proj, np.float32)
    b_proj = np.asarray(b_proj, np.float32)
    P = x.shape[0]
    xf = np.ascontiguousarray(x.reshape(P, C, N))
    wqT = _bf16(np.ascontiguousarray((w_qkv[0:C] * SCALE).T))
    wkT = _bf16(np.ascontiguousarray(w_qkv[C : 2 * C].T))
    wvT = _bf16(np.ascontiguousarray(w_qkv[2 * C : 3 * C].T))
    wpT = _bf16(np.ascontiguousarray(w_proj.T))
    bias = np.ascontiguousarray(b_proj.reshape(C, 1))
    in_maps = []
    for core in range(N_CORES):
        p, mh = divmod(core, 2)
        if mh == 0:
            xp = xf[p]
        else:
            # rotate the n axis so this core's query half comes first
            xp = np.concatenate([xf[p][:, M:], xf[p][:, :M]], axis=1)
        in_maps.append(
            {
                "xf": _bf16(np.ascontiguousarray(xp)),
                "wqT": wqT,
                "wkT": wkT,
                "wvT": wvT,
                "wpT": wpT,
                "bias": bias,
            }
        )
    return in_maps


def assemble_output(per_core_y, x_shape):
    P, B, _, H, W, D = x_shape
    y = np.empty((P, C, N), np.float32)
    for core in range(N_CORES):
        p, mh = divmod(core, 2)
        y[p][:, mh * M : (mh + 1) * M] = per_core_y[core]
    return y.reshape(P, B, C, H, W, D)


def kernel(x, w_qkv, w_proj, b_proj):
    nc = _get_nc()
    in_maps = make_in_maps(x, w_qkv, w_proj, b_proj)
    res = run_bass_kernel_spmd(nc, in_maps, core_ids=list(range(N_CORES)))
    return assemble_output([res.results[c]["y"] for c in range(N_CORES)], x.shape)
